# revision 27
# baseline (speedup 1.0000x reference)
"""Trainium2 Bass kernel for a dense transformer decoder block.

Reference computation (B=4, S=2048, D=768, H=12, DK=64, DF=3072):
    q,k,v = x@wq+bq, x@wk+bk, x@wv+bv          (per-head split, DK=64)
    attn  = softmax(mask(q k^T / 8))
    ctx   = attn @ v
    h     = LN(ctx@wo + bo + x; g1, be1)
    out   = LN(gelu_exact(h@w1 + b1)@w2 + b2 + h; g2, be2)

Sharding: pure data parallel, zero collectives. 8 cores = 4 batch elements
x 2 query groups of 1024 rows. Queries are snake-paired in 256-row blocks
(core 2b+0 gets absolute blocks {0,3,4,7}, core 2b+1 gets {1,2,5,6}) so the
four query slots need key extents [4,8,12,16] key-blocks of 128 on BOTH
cores of a pair -- exact block-causal coverage with zero extent waste, and
only the last 4 key blocks of each slot carry mask data (applied as data).
Every core runs the identical SPMD program; per-core behavior differs only
through input data (sliced/transposed/cast on the host).

Schedule: attention is ACT-(exp)-bound, so independent PE work is woven
between attention iterations to keep the tensor engine dense (and its HAM
clock warm): the sb2/sb3 K,V projections run under qb0 attention, and the
qb0 out-projection + LN1 + h-transposes run under qb1 attention.

v2 refinements (engine rebalance around the ACT exp chain):
- LayerNorm rstd via one batched ACT Rsqrt per group of 4 LNs (eliminates
  the per-LN Sqrt + DVE reciprocal and the exp<->sqrt ACT-table thrash).
- All PSUM->SBUF epilogues woven into attention windows run on DVE, not
  ACT (k/v projection bias/copy, h-transpose copies, batched per 3 chunks).
- Softmax denominator reciprocal via reciprocal_approx_fast (~5x faster).
- DMA order matches consumption: wq/wk/wv arrive per consumed column block,
  mask half mT1 moves out of the startup burst, and the FFN weights w1/w2
  start streaming before the qc4-7 epilogue so FFN1 never waits on HBM.
"""

from contextlib import ExitStack

import numpy as np
import ml_dtypes

import concourse.bass as bass
import concourse.tile as tile
from concourse import bacc, mybir
from concourse.bass_utils import run_bass_kernel_spmd
from concourse.masks import make_identity

F32 = mybir.dt.float32
BF16 = mybir.dt.bfloat16
AF = mybir.ActivationFunctionType
OP = mybir.AluOpType
BF = ml_dtypes.bfloat16

B, S, D, H, DK = 4, 2048, 768, 12, 64
DF = 4 * D
EPS = 1e-5
P = 128
SQ = 1024            # query rows per core
HP = H // 2          # 6 head pairs
KB = S // P          # 16 key blocks
QB = 2               # query halves of 512 per core (projection/FFN granularity)
QBS = 512
SS = 256             # attention query-slot size
SLOTS = 4            # slots per core; snake block pairing equalizes extents
EXTS = [4, 8, 12, 16]  # exact key-block extent per slot (block-causal skip)
MW = 4               # mask window: only the last MW key blocks of a slot mask
DC = D // P          # 6 chunks of the model dim
DFC = DF // P        # 24 chunks of the FFN dim
QC = SQ // P         # 8 query chunks of 128
NH = 2               # 384-wide halves of D for PSUM-friendly matmul N
NHW = D // NH        # 384
SB = S // QBS        # 4 key column slabs

N_CORES = 8


def emit(ctx: ExitStack, tc: tile.TileContext, io: dict):
    nc = tc.nc

    xT, xqT, xres, maskT = io["xT"], io["xqT"], io["xres"], io["maskT"]
    wq, wk, wv, wo, w1, w2 = io["wq"], io["wk"], io["wv"], io["wo"], io["w1"], io["w2"]
    out = io["out"]

    # ---- constants ----------------------------------------------------
    const = ctx.enter_context(tc.tile_pool(name="const", bufs=1))
    ident = const.tile([P, P], BF16)
    make_identity(nc, ident)
    eps_t = const.tile([P, 1], F32)
    nc.vector.memset(eps_t, EPS)

    bqp = const.tile([P, HP], F32)
    nc.gpsimd.dma_start(out=bqp, in_=io["bqp"])
    bkp = const.tile([P, HP], F32)
    nc.gpsimd.dma_start(out=bkp, in_=io["bkp"])
    b1p = const.tile([P, DFC], F32)
    nc.gpsimd.dma_start(out=b1p, in_=io["b1p"])

    def brow(name):
        # [1, D] dram tensor broadcast-DMA'd across 128 partitions
        t = const.tile([P, D], F32, tag=name)
        a = io[name]
        src = bass.AP(tensor=a.tensor, offset=a.offset, ap=[[0, P]] + list(a.ap[1:]))
        nc.gpsimd.dma_start(out=t, in_=src)
        return t

    g1b, be1b, g2b, be2b, b2b = map(brow, ["g1r", "be1r", "g2r", "be2r", "b2r"])

    # ---- FFN-phase tensors: left stack, below attn_in so release order
    # stays LIFO (h/hT are written during the attention epilogue fillers)
    ffn = tc.alloc_tile_pool(name="ffn", bufs=1)
    h_sb = ffn.tile([P, QC, D], BF16)     # LN1 out (residual + FFN rhs)
    hT = ffn.tile([P, DC, SQ], BF16)
    ln_wk = tc.alloc_tile_pool(name="ln_wk", bufs=1)

    # ---- attention inputs (live through attention) --------------------
    attn_in = tc.alloc_tile_pool(name="attn_in", bufs=1)
    KT = attn_in.tile([P, HP, S], BF16)            # K^T, head pairs on partitions
    Vaug = attn_in.tile([P, KB, H, DK + 1], BF16)  # V + ones column per head
    QT = attn_in.tile([P, HP, SQ], BF16)
    # only the mask quadrants that aren't structurally trivial: with the
    # snake slot pairing, just the last MW key blocks of each slot can hold
    # a diagonal (or an all-masked block) on some core
    mS = attn_in.tile([P, SLOTS, MW, SS], BF16)
    mr = maskT.rearrange("(kb p) q -> p kb q", p=P)
    nc.vector.memset(Vaug[:, :, :, DK : DK + 1], 1.0)

    # ---- post-attention inputs (right-side stack, phase-scoped) -------
    mid_ctx = tc.alloc_tile_pool(name="mid_ctx", bufs=1, side="right")
    ctxT = mid_ctx.tile([P, DC, SQ], BF16)

    kv_in = tc.alloc_tile_pool(name="kv_in", bufs=1, side="right")
    wk_sb = kv_in.tile([P, DC, D], BF16)
    wv_sb = kv_in.tile([P, DC, D], BF16)
    xT23 = kv_in.tile([P, DC, S // 2], BF16)
    xt01p = tc.alloc_tile_pool(name="xt01p", bufs=1, side="right")
    xT01 = xt01p.tile([P, DC, S // 2], BF16)
    xTr = xT.rearrange("(c p) s -> p c s", p=P)

    def xT_at(sb):
        t = xT01 if sb < 2 else xT23
        return t, (sb % 2) * QBS

    q_in = tc.alloc_tile_pool(name="q_in", bufs=1, side="right")
    wq_sb = q_in.tile([P, DC, D], BF16)
    xqT_sb = q_in.tile([P, DC, SQ], BF16)
    # DMA order tracks first consumption (q projections, then K slab 0, ...).
    # Weight tensors go as whole contiguous transfers: slicing them per
    # column block explodes the descriptor-issue time on the queue engine.
    wkr = wk.rearrange("(c p) n -> p c n", p=P)
    wvr = wv.rearrange("(c p) n -> p c n", p=P)
    xqr = xqT.rearrange("(c p) s -> p c s", p=P)
    nc.sync.dma_start(out=wq_sb, in_=wq.rearrange("(c p) n -> p c n", p=P))
    nc.scalar.dma_start(out=xqT_sb[:, :, 0:QBS], in_=xqr[:, :, 0:QBS])
    nc.sync.dma_start(out=xT01[:, :, 0:QBS], in_=xTr[:, :, 0:QBS])
    for c in range(DC):
        nc.scalar.dma_start(out=wk_sb[:, c, :], in_=wkr[:, c, :])
    nc.sync.dma_start(out=xT01[:, :, QBS : 2 * QBS], in_=xTr[:, :, QBS : 2 * QBS])
    nc.scalar.dma_start(out=xqT_sb[:, :, QBS : 2 * QBS],
                        in_=xqr[:, :, QBS : 2 * QBS])
    for c in range(DC):
        nc.gpsimd.dma_start(out=wv_sb[:, c, :], in_=wvr[:, c, :])
    for sb in (2, 3):
        nc.gpsimd.dma_start(out=xT23[:, :, (sb % 2) * QBS : (sb % 2 + 1) * QBS],
                            in_=xTr[:, :, sb * QBS : (sb + 1) * QBS])

    # ---- LayerNorm helpers (stats on DVE, rstd batched on ACT) --------
    def ln_stats(src, mvs, j):
        # bn stats over the free dim (768) of fp32 src [128, 768]
        stats = ln_wk.tile([P, 3, 6], F32, tag="stats", bufs=3)
        for i in range(3):
            nc.vector.bn_stats(out=stats[:, i, :], in_=src[:, i * 256 : (i + 1) * 256])
        nc.vector.bn_aggr(out=mvs[:, j, :], in_=stats)

    def ln_rstd_batch(mvs, rstds, n):
        # one ACT Sqrt + one fast DVE reciprocal serve n LayerNorms
        stds = ln_wk.tile([P, 4], F32, tag="stds", bufs=2)
        nc.scalar.activation(out=stds[:, 0:n], in_=mvs[:, 0:n, 1:2],
                             func=AF.Sqrt, bias=eps_t[:, 0:1])
        nc.vector.reciprocal_approx_fast(out=rstds[:, 0:n], in_=stds[:, 0:n])

    def ln_apply(src, mvs, rstds, j, gb, bb, dst):
        nc.vector.tensor_scalar_sub(out=src, in0=src, scalar1=mvs[:, j, 0:1])
        nc.vector.scalar_tensor_tensor(out=src, in0=src, scalar=rstds[:, j : j + 1],
                                       in1=gb, op0=OP.mult, op1=OP.mult)
        nc.vector.tensor_tensor(out=dst, in0=src, in1=bb, op=OP.add)

    proj_ps = tc.alloc_tile_pool(name="proj_ps", bufs=2, space="PSUM", side="right")
    if True:
        sc_ps = tc.alloc_tile_pool(name="sc_ps", bufs=2, space="PSUM")
        cx_ps = tc.alloc_tile_pool(name="cx_ps", bufs=1, space="PSUM")
        # at_sb/nm_sb/mT0 are allocated only once the q-projection inputs are
        # released -- their SBUF footprints must not overlap
        pools = {}

        # ---------- projection work units ----------
        def q_unit(hp, sb):
            ps = proj_ps.tile([P, QBS], F32, tag="proj")
            for c in range(DC):
                nc.tensor.matmul(
                    ps, lhsT=wq_sb[:, c, hp * P : (hp + 1) * P],
                    rhs=xqT_sb[:, c, sb * QBS : (sb + 1) * QBS],
                    start=(c == 0), stop=(c == DC - 1),
                )
            nc.scalar.activation(
                out=QT[:, hp, sb * QBS : (sb + 1) * QBS], in_=ps,
                func=AF.Identity, bias=bqp[:, hp : hp + 1],
            )

        def k_unit(hp, sb, on_act=True):
            xt, off = xT_at(sb)
            ps = proj_ps.tile([P, QBS], F32, tag="proj")
            for c in range(DC):
                nc.tensor.matmul(
                    ps, lhsT=wk_sb[:, c, hp * P : (hp + 1) * P],
                    rhs=xt[:, c, off : off + QBS],
                    start=(c == 0), stop=(c == DC - 1),
                )
            if on_act:
                nc.scalar.activation(
                    out=KT[:, hp, sb * QBS : (sb + 1) * QBS], in_=ps,
                    func=AF.Identity, bias=bkp[:, hp : hp + 1],
                )
            else:
                # inside the attention interleave ACT is the bottleneck chain
                nc.vector.tensor_scalar_add(
                    out=KT[:, hp, sb * QBS : (sb + 1) * QBS], in0=ps,
                    scalar1=bkp[:, hp : hp + 1],
                )

        def v_unit(kb, nh, on_act=True):
            xt, off = xT_at(kb // (QBS // P))
            kb_off = off // P + kb % (QBS // P)
            ps = proj_ps.tile([P, QBS], F32, tag="proj")
            psv = ps[:, 0:NHW]
            for c in range(DC):
                nc.tensor.matmul(
                    psv, lhsT=xt[:, c, kb_off * P : (kb_off + 1) * P],
                    rhs=wv_sb[:, c, nh * NHW : (nh + 1) * NHW],
                    start=(c == 0), stop=(c == DC - 1),
                )
            if on_act:
                nc.scalar.activation(
                    out=Vaug[:, kb, nh * 6 : (nh + 1) * 6, 0:DK],
                    in_=psv.rearrange("p (h d) -> p h d", d=DK),
                    func=AF.Copy,
                )
            else:
                nc.vector.tensor_copy(
                    out=Vaug[:, kb, nh * 6 : (nh + 1) * 6, 0:DK],
                    in_=psv.rearrange("p (h d) -> p h d", d=DK),
                )

        def kv_slab(sb, on_act=True):
            for hp in range(HP):
                k_unit(hp, sb, on_act)
            for j in range(QBS // P):
                for nh in range(NH):
                    v_unit(sb * (QBS // P) + j, nh, on_act)

        # ---------- attention iteration ----------
        pending = []

        def make_norm(cxs_e, cxs_o, rec_p, hp, qs):
            def go():
                for i, (cxs, pb) in enumerate(((cxs_e, 0), (cxs_o, DK))):
                    den_b = pools['nm_sb'].tile([DK, SS], F32, tag="den_b", bufs=1)
                    nc.gpsimd.partition_broadcast(den_b, rec_p[0:1, i, :])
                    nc.vector.tensor_tensor(
                        out=ctxT[pb : pb + DK, hp, qs], in0=cxs,
                        in1=den_b, op=OP.mult,
                    )
            return go

        def attn_iter(hp, s, fill=None):
            ext = EXTS[s]
            qs = slice(s * SS, (s + 1) * SS)
            # cx/sc tiles are padded to a PSUM bank per concurrent writer:
            # the paired head matmuls write different banks, and each
            # accumulation group owns its bank (first_mm clears a full bank)
            cx_e_t = cx_ps.tile([DK + 1, 2 * SS], F32, tag="cx_e", name="cx_e")
            cx_o_t = cx_ps.tile([DK + 1, 2 * SS], F32, tag="cx_o", name="cx_o")
            cx_e = cx_e_t[:, 0:SS]
            cx_o = cx_o_t[:, 0:SS]
            for gb in range(0, ext, 2):
                pt = pools['at_sb'].tile([P, 2, 2, SS], BF16, tag="pt")
                for gi in range(2):
                    g = gb + gi
                    ks = slice(g * P, (g + 1) * P)
                    sc = sc_ps.tile([P, 2, 2 * SS], F32, tag="sc")
                    # the two heads of a pair hit disjoint PE row groups and
                    # run concurrently in the array
                    nc.tensor.matmul(sc[:, 0, 0:SS], lhsT=KT[0:DK, hp, ks],
                                     rhs=QT[0:DK, hp, qs], start=True, stop=True)
                    nc.tensor.matmul(sc[:, 1, 0:SS], lhsT=KT[DK:P, hp, ks],
                                     rhs=QT[DK:P, hp, qs], start=True, stop=True)
                    nc.scalar.activation(out=pt[:, gi, :, :], in_=sc[:, :, 0:SS],
                                         func=AF.Exp, scale=1.0 / 8.0)
                # only the last MW key blocks of a slot carry mask data
                if gb >= ext - MW:
                    mq = mS[:, s, gb - (ext - MW) : gb - (ext - MW) + 2, :]
                    for hh in range(2):
                        nc.vector.tensor_tensor(
                            out=pt[:, :, hh, :], in0=pt[:, :, hh, :],
                            in1=mq, op=OP.mult,
                        )
                for gi in range(2):
                    g = gb + gi
                    nc.tensor.matmul(cx_e, lhsT=Vaug[:, g, 2 * hp, :],
                                     rhs=pt[:, gi, 0, :],
                                     start=(g == 0), stop=(g == ext - 1))
                    nc.tensor.matmul(cx_o, lhsT=Vaug[:, g, 2 * hp + 1, :],
                                     rhs=pt[:, gi, 1, :],
                                     start=(g == 0), stop=(g == ext - 1))
                if gb == 2 and pending:
                    # previous iteration's normalize: emitted after this
                    # iteration's first blocks so the DVE reciprocal never
                    # delays the mask multiplies
                    pending.pop()()
                # the PE is in-order: filler matmuls only absorb the exp-wait
                # bubbles if they are woven BETWEEN key-block groups
                if fill and (s <= 1 or gb % 4 == 2):
                    fill.pop(0)()
            # stage ctx to SBUF immediately: frees the PSUM bank within one
            # DVE copy so the cx pool gets away with a single buffer; the
            # denominator rows pair up in one base-0 tile for one reciprocal
            cxs_e = pools['nm_sb'].tile([DK, SS], F32, tag="cxs_e")
            nc.vector.tensor_copy(out=cxs_e, in_=cx_e[0:DK, :])
            cxs_o = pools['nm_sb'].tile([DK, SS], F32, tag="cxs_o")
            nc.vector.tensor_copy(out=cxs_o, in_=cx_o[0:DK, :])
            den_p = pools['nm_sb'].tile([1, 2, SS], F32, tag="den_p")
            nc.vector.tensor_copy(out=den_p[:, 0, :], in_=cx_e[DK : DK + 1, :])
            nc.vector.tensor_copy(out=den_p[:, 1, :], in_=cx_o[DK : DK + 1, :])
            rec_p = pools['nm_sb'].tile([1, 2, SS], F32, tag="rec_p")
            nc.vector.reciprocal_approx_fast(out=rec_p, in_=den_p)
            pending.append(make_norm(cxs_e, cxs_o, rec_p, hp, qs))

        # ---------- schedule: projections + qb0 attention ----------
        for hp in range(HP):
            q_unit(hp, 0)
        kv_slab(0)
        kv_slab(1)
        for hp in range(HP):
            q_unit(hp, 1)
        q_in.release()
        xt01p.release()
        pools['at_sb'] = tc.alloc_tile_pool(name="at_sb", bufs=2)
        pools['nm_sb'] = tc.alloc_tile_pool(name="nm_sb", bufs=2)
        # mask windows load after the startup burst, in the early attention
        # window's spare DMA bandwidth (slot s is consumed ~s quarters in)
        for s in range(SLOTS):
            nc.gpsimd.dma_start(
                out=mS[:, s, :, :],
                in_=mr[:, EXTS[s] - MW : EXTS[s], s * SS : (s + 1) * SS])
        # epilogues of woven fillers split across ACT and DVE to balance the
        # two against the exp chain (ACT) and mask multiplies (DVE)
        kv_fill = [(lambda hp=hp, sb=sb: k_unit(hp, sb, on_act=False))
                   for sb in (2, 3) for hp in range(HP)] + \
                  [(lambda kb=kb, nh=nh: v_unit(kb, nh, on_act=(nh == 0)))
                   for kb in range(8, KB) for nh in range(NH)]
        for s in (0, 1):
            for hp in range(HP):
                attn_iter(hp, s, kv_fill)
        for fn in kv_fill:
            fn()
        kv_fill.clear()
        kv_in.release()
        proj_ps.release()

        # ---------- qb1 attention with qb0 epilogue woven in ----------
        mid_ow = tc.alloc_tile_pool(name="mid_ow", bufs=1, side="right")
        xres_sb = mid_ow.tile([P, QC, D], BF16)
        nc.gpsimd.dma_start(out=xres_sb,
                            in_=xres.rearrange("(c p) n -> p c n", p=P))
        wo_sb = mid_ow.tile([P, DC, D], BF16)
        nc.gpsimd.dma_start(out=wo_sb, in_=wo.rearrange("(c p) n -> p c n", p=P))
        op_ps = tc.alloc_tile_pool(name="op_ps", bufs=1, space="PSUM", side="right")
        tp_ps = tc.alloc_tile_pool(name="tp_ps", bufs=1, space="PSUM", side="right")

        hpre_map = {}

        def op_half(qc, nh):
            def go():
                if qc not in hpre_map:
                    hpre_map[qc] = ln_wk.tile([P, D], F32, tag="hpre",
                                              bufs=4, name=f"hpre_{qc}")
                hpre = hpre_map[qc]
                ps = op_ps.tile([P, NHW], F32, tag="op")
                for c in range(DC):
                    nc.tensor.matmul(
                        ps, lhsT=ctxT[:, c, qc * P : (qc + 1) * P],
                        rhs=wo_sb[:, c, nh * NHW : (nh + 1) * NHW],
                        start=(c == 0), stop=(c == DC - 1),
                    )
                nc.vector.scalar_tensor_tensor(
                    out=hpre[:, nh * NHW : (nh + 1) * NHW], in0=ps,
                    scalar=1.0, in1=xres_sb[:, qc, nh * NHW : (nh + 1) * NHW],
                    op0=OP.mult, op1=OP.add,
                )
            return go

        def ln1_group(qcs):
            # per-group LN state: stats land in mvs, one ACT computes all
            # rstds, applies run per qc
            mvs = ln_wk.tile([P, 4, 2], F32, tag="mvs", bufs=2)
            rstds = ln_wk.tile([P, 4], F32, tag="rstds", bufs=2)

            def st(j):
                def go():
                    ln_stats(hpre_map[qcs[j]], mvs, j)
                return go

            def rs():
                def go():
                    ln_rstd_batch(mvs, rstds, len(qcs))
                return go

            def ap(j):
                def go():
                    ln_apply(hpre_map.pop(qcs[j]), mvs, rstds, j, g1b, be1b,
                             h_sb[:, qcs[j], :])
                return go

            return [st(j) for j in range(len(qcs))], rs(), \
                   [ap(j) for j in range(len(qcs))]

        def transp_half(qc, lo):
            def go():
                tp = tp_ps.tile([P, DC // 2, P], BF16, tag="tp")
                for i, c in enumerate(range(lo, lo + DC // 2)):
                    nc.tensor.transpose(tp[:, i, :],
                                        h_sb[:, qc, c * P : (c + 1) * P], ident)
                nc.vector.tensor_copy(
                    out=hT[:, lo : lo + DC // 2, qc * P : (qc + 1) * P], in_=tp)
            return go

        st03, rs03, ap03 = ln1_group([0, 1, 2, 3])
        fillers = []
        for qc in range(4):
            fillers += [op_half(qc, 0), op_half(qc, 1), st03[qc]]
        fillers.append(rs03)
        fillers += ap03
        for qc in range(4):
            fillers += [transp_half(qc, 0), transp_half(qc, DC // 2)]
        for s in (2, 3):
            for hp in range(HP):
                attn_iter(hp, s, fillers)
        for fn in pending:
            fn()
        pending.clear()
        for fn in fillers:
            fn()

        # ---------- attention PSUM closes; FFN weights start streaming NOW
        # (9.2MB issued before the qc4-7 epilogue so FFN1 never stalls) ----
        cx_ps.release()
        sc_ps.release()
        pools['nm_sb'].release()
        pools['at_sb'].release()
        attn_in.release()
        w12_in = tc.alloc_tile_pool(name="w12_in", bufs=1)
        w1_sb = w12_in.tile([P, DC, DF], BF16)
        w2_sb = w12_in.tile([P, DFC, D], BF16)
        w1r = w1.rearrange("(c p) n -> p c n", p=P)
        for fq in range(4):
            nc.sync.dma_start(out=w1_sb[:, :, fq * (DF // 4) : (fq + 1) * (DF // 4)],
                              in_=w1r[:, :, fq * (DF // 4) : (fq + 1) * (DF // 4)])
        nc.scalar.dma_start(out=w2_sb, in_=w2.rearrange("(c p) n -> p c n", p=P))
        f1_ps = tc.alloc_tile_pool(name="f1_ps", bufs=3, space="PSUM")
        f1g_sb = tc.alloc_tile_pool(name="f1g_sb", bufs=1)

        def ffn1_unit(f1g, qs, f):
            ps = f1_ps.tile([P, QBS], F32, tag="f1")
            for c in range(DC):
                nc.tensor.matmul(
                    ps, lhsT=w1_sb[:, c, f * P : (f + 1) * P],
                    rhs=hT[:, c, qs], start=(c == 0), stop=(c == DC - 1),
                )
            nc.scalar.activation(out=f1g[:, f, :], in_=ps, func=AF.Gelu,
                                 bias=b1p[:, f : f + 1])

        # ---------- rest of out-projection + LN1 + transposes, interleaved
        # with FFN1 on the first query block ----------
        st47, rs47, ap47 = ln1_group([4, 5, 6, 7])
        epi = []
        for j, qc in enumerate(range(4, QC)):
            epi += [op_half(qc, 0), op_half(qc, 1), st47[j]]
        epi.append(rs47)
        epi += ap47
        for qc in range(4, QC):
            epi += [transp_half(qc, 0), transp_half(qc, DC // 2)]
        f1g0 = f1g_sb.tile([P, DFC, QBS], BF16, tag="f1g")
        for f in range(DFC):
            if epi:
                epi.pop(0)()
            ffn1_unit(f1g0, slice(0, QBS), f)
        for fn in epi:
            fn()

    tp_ps.release()
    op_ps.release()
    mid_ow.release()
    mid_ctx.release()

    # ====== FFN: f1^T = gelu(w1^T h^T + b1); out = LN2(f1g^T w2 + h) ====
    with tc.tile_pool(name="f2_ps", bufs=3, space="PSUM") as f2_ps, \
         tc.tile_pool(name="out_sb", bufs=3) as out_sb:
        def ffn2_qc(f1g, qb, sq):
            qc = qb * (QBS // P) + sq
            ot = out_sb.tile([P, D], F32, tag="ot")
            for nh in range(NH):
                ps = f2_ps.tile([P, NHW], F32, tag="f2")
                for f in range(DFC):
                    nc.tensor.matmul(
                        ps, lhsT=f1g[:, f, sq * P : (sq + 1) * P],
                        rhs=w2_sb[:, f, nh * NHW : (nh + 1) * NHW],
                        start=(f == 0), stop=(f == DFC - 1),
                    )
                nc.vector.scalar_tensor_tensor(
                    out=ot[:, nh * NHW : (nh + 1) * NHW], in0=ps, scalar=1.0,
                    in1=h_sb[:, qc, nh * NHW : (nh + 1) * NHW],
                    op0=OP.mult, op1=OP.add,
                )
            nc.vector.tensor_tensor(out=ot, in0=ot, in1=b2b, op=OP.add)
            mvs2 = ln_wk.tile([P, 4, 2], F32, tag="mvs", bufs=2)
            rstds2 = ln_wk.tile([P, 4], F32, tag="rstds", bufs=2)
            ln_stats(ot, mvs2, 0)
            ln_rstd_batch(mvs2, rstds2, 1)
            ln_apply(ot, mvs2, rstds2, 0, g2b, be2b, ot)
            nc.sync.dma_start(out=out[qc * P : (qc + 1) * P, :], in_=ot)

        for sq in range(QBS // P):
            ffn2_qc(f1g0, 0, sq)
        f1g1 = f1g_sb.tile([P, DFC, QBS], BF16, tag="f1g")
        for f in range(DFC):
            ffn1_unit(f1g1, slice(QBS, 2 * QBS), f)
        for sq in range(QBS // P):
            ffn2_qc(f1g1, 1, sq)

    f1_ps.release()
    f1g_sb.release()
    w12_in.release()
    ln_wk.release()
    ffn.release()


def build_program():
    nc = bacc.Bacc("TRN2", target_bir_lowering=False, debug=False,
                   enable_asserts=False, num_devices=N_CORES)
    io = {}

    def din(name, shape, dt):
        io[name] = nc.dram_tensor(name, list(shape), dt, kind="ExternalInput").ap()

    din("xT", (D, S), BF16)
    din("xqT", (D, SQ), BF16)
    din("xres", (SQ, D), BF16)
    din("maskT", (S, SQ), BF16)
    din("wq", (D, D), BF16)
    din("wk", (D, D), BF16)
    din("wv", (D, D), BF16)
    din("wo", (D, D), BF16)
    din("w1", (D, DF), BF16)
    din("w2", (DF, D), BF16)
    din("bqp", (P, HP), F32)
    din("bkp", (P, HP), F32)
    din("b1p", (P, DFC), F32)
    for n in ["g1r", "be1r", "g2r", "be2r", "b2r"]:
        din(n, (1, D), F32)
    io["out"] = nc.dram_tensor("out", [SQ, D], F32, kind="ExternalOutput").ap()

    with tile.TileContext(nc) as tc:
        with ExitStack() as ctx:
            emit(ctx, tc, io)
    nc.compile()
    return nc


_NC = None


def _get_program():
    global _NC
    if _NC is None:
        _NC = build_program()
    return _NC


def _qrows(half):
    # snake pairing of 256-row blocks: slot extents become [4,8,12,16] key
    # blocks on BOTH cores of a pair (exact block-causal coverage, SPMD-safe)
    blocks = [0, 3, 4, 7] if half == 0 else [1, 2, 5, 6]
    return np.concatenate([np.arange(256 * a, 256 * (a + 1)) for a in blocks])


def shard_inputs(inputs):
    x = np.asarray(inputs["x"], np.float32)
    mask = np.asarray(inputs["mask"], bool)
    w = {k: np.asarray(inputs[k], np.float32) for k in
         ["wq", "bq", "wk", "bk", "wv", "bv", "wo", "bo", "g1", "be1",
          "w1", "b1", "w2", "b2", "g2", "be2"]}

    base = dict(
        wq=np.ascontiguousarray(w["wq"].astype(BF)),
        wk=np.ascontiguousarray(w["wk"].astype(BF)),
        wv=np.ascontiguousarray(w["wv"].astype(BF)),
        wo=np.ascontiguousarray(w["wo"].astype(BF)),
        w1=np.ascontiguousarray(w["w1"].astype(BF)),
        w2=np.ascontiguousarray(w["w2"].astype(BF)),
        bqp=np.ascontiguousarray(w["bq"].reshape(HP, P).T),
        bkp=np.ascontiguousarray(w["bk"].reshape(HP, P).T),
        b1p=np.ascontiguousarray(w["b1"].reshape(DFC, P).T),
        g1r=np.ascontiguousarray(w["g1"].reshape(1, D)),
        be1r=np.ascontiguousarray(w["be1"].reshape(1, D)),
        g2r=np.ascontiguousarray(w["g2"].reshape(1, D)),
        be2r=np.ascontiguousarray(w["be2"].reshape(1, D)),
        b2r=np.ascontiguousarray(w["b2"].reshape(1, D)),
    )
    # bv and bo fold into the residual: ctx@wo + bo + x with v-bias bv adds
    # a constant row bv@wo (softmax rows sum to 1)
    res_const = (w["bo"] + w["bv"] @ w["wo"]).astype(np.float32)

    in_maps = []
    for c in range(N_CORES):
        b, half = divmod(c, 2)
        qr = _qrows(half)
        xb = x[b]
        xq = xb[qr]
        m = dict(base)
        m["xT"] = np.ascontiguousarray(xb.T.astype(BF))
        m["xqT"] = np.ascontiguousarray(xq.T.astype(BF))
        m["xres"] = np.ascontiguousarray((xq + res_const[None, :]).astype(BF))
        m["maskT"] = np.ascontiguousarray((~mask[b][qr]).T.astype(BF))
        in_maps.append(m)
    return in_maps


def gather_outputs(results):
    y = np.empty((B, S, D), np.float32)
    for c in range(N_CORES):
        b, half = divmod(c, 2)
        y[b, _qrows(half)] = results[c]["out"]
    return y


def kernel(**inputs):
    nc = _get_program()
    in_maps = shard_inputs(inputs)
    res = run_bass_kernel_spmd(nc, in_maps, list(range(N_CORES)))
    return gather_outputs(res.results)


if __name__ == "__main__":
    build_program()
    print("program built ok")


# revision 31
# speedup vs baseline: 1.1017x; 1.1017x over previous
"""Trainium2 Bass kernel for a dense transformer decoder block.

Reference computation (B=4, S=2048, D=768, H=12, DK=64, DF=3072):
    q,k,v = x@wq+bq, x@wk+bk, x@wv+bv          (per-head split, DK=64)
    attn  = softmax(mask(q k^T / 8))
    ctx   = attn @ v
    h     = LN(ctx@wo + bo + x; g1, be1)
    out   = LN(gelu_exact(h@w1 + b1)@w2 + b2 + h; g2, be2)

Sharding: pure data parallel, zero collectives. 8 cores = 4 batch elements
x 2 query groups of 1024 rows. Queries are snake-paired in 256-row blocks
(core 2b+0 gets absolute blocks {0,3,4,7}, core 2b+1 gets {1,2,5,6}) so the
four query slots need key extents [4,8,12,16] key-blocks of 128 on BOTH
cores of a pair -- exact block-causal coverage with zero extent waste, and
only the last 4 key blocks of each slot carry mask data (applied as data).
Every core runs the identical SPMD program; per-core behavior differs only
through input data (sliced/transposed/cast on the host).

Schedule: attention is ACT-(exp)-bound, so independent PE work is woven
between attention iterations to keep the tensor engine dense (and its HAM
clock warm): the sb2/sb3 K,V projections run under qb0 attention, and the
qb0 out-projection + LN1 + h-transposes run under qb1 attention.

v2 refinements (engine rebalance around the ACT exp chain):
- LayerNorm rstd via one batched ACT Rsqrt per group of 4 LNs (eliminates
  the per-LN Sqrt + DVE reciprocal and the exp<->sqrt ACT-table thrash).
- All PSUM->SBUF epilogues woven into attention windows run on DVE, not
  ACT (k/v projection bias/copy, h-transpose copies, batched per 3 chunks).
- Softmax denominator reciprocal via reciprocal_approx_fast (~5x faster).
- DMA order matches consumption: wq/wk/wv arrive per consumed column block,
  mask half mT1 moves out of the startup burst, and the FFN weights w1/w2
  start streaming before the qc4-7 epilogue so FFN1 never waits on HBM.
"""

from contextlib import ExitStack

import numpy as np
import ml_dtypes

import concourse.bass as bass
import concourse.tile as tile
from concourse import bacc, mybir
from concourse.bass_utils import run_bass_kernel_spmd
from concourse.masks import make_identity

F32 = mybir.dt.float32
BF16 = mybir.dt.bfloat16
AF = mybir.ActivationFunctionType
OP = mybir.AluOpType
BF = ml_dtypes.bfloat16

B, S, D, H, DK = 4, 2048, 768, 12, 64
DF = 4 * D
EPS = 1e-5
P = 128
SQ = 1024            # query rows per core
HP = H // 2          # 6 head pairs
KB = S // P          # 16 key blocks
QB = 2               # query halves of 512 per core (projection/FFN granularity)
QBS = 512
SS = 256             # attention query-slot size
SLOTS = 4            # slots per core; snake block pairing equalizes extents
EXTS = [4, 8, 12, 16]  # exact key-block extent per slot (block-causal skip)
MW = 4               # mask window: only the last MW key blocks of a slot mask
DC = D // P          # 6 chunks of the model dim
DFC = DF // P        # 24 chunks of the FFN dim
QC = SQ // P         # 8 query chunks of 128
NH = 2               # 384-wide halves of D for PSUM-friendly matmul N
NHW = D // NH        # 384
SB = S // QBS        # 4 key column slabs

N_CORES = 8


def emit(ctx: ExitStack, tc: tile.TileContext, io: dict):
    nc = tc.nc

    xT, xqT, xres, maskT = io["xT"], io["xqT"], io["xres"], io["maskT"]
    wq, wk, wv, wo, w1, w2 = io["wq"], io["wk"], io["wv"], io["wo"], io["w1"], io["w2"]
    out = io["out"]

    # ---- constants ----------------------------------------------------
    const = ctx.enter_context(tc.tile_pool(name="const", bufs=1))
    ident = const.tile([P, P], BF16)
    make_identity(nc, ident)
    eps_t = const.tile([P, 1], F32)
    nc.vector.memset(eps_t, EPS)

    bqp = const.tile([P, HP], F32)
    nc.gpsimd.dma_start(out=bqp, in_=io["bqp"])
    bkp = const.tile([P, HP], F32)
    nc.gpsimd.dma_start(out=bkp, in_=io["bkp"])
    b1p = const.tile([P, DFC], F32)
    nc.gpsimd.dma_start(out=b1p, in_=io["b1p"])

    def brow(name):
        # [1, D] dram tensor broadcast-DMA'd across 128 partitions
        t = const.tile([P, D], F32, tag=name)
        a = io[name]
        src = bass.AP(tensor=a.tensor, offset=a.offset, ap=[[0, P]] + list(a.ap[1:]))
        nc.gpsimd.dma_start(out=t, in_=src)
        return t

    g1b, be1b, g2b, be2b, b2b = map(brow, ["g1r", "be1r", "g2r", "be2r", "b2r"])

    # ---- FFN-phase tensors: left stack, below attn_in so release order
    # stays LIFO (h/hT are written during the attention epilogue fillers)
    ffn = tc.alloc_tile_pool(name="ffn", bufs=1)
    h_sb = ffn.tile([P, QC, D], BF16)     # LN1 out (residual + FFN rhs)
    hT = ffn.tile([P, DC, SQ], BF16)
    ln_wk = tc.alloc_tile_pool(name="ln_wk", bufs=1)

    # ---- attention inputs (live through attention) --------------------
    attn_in = tc.alloc_tile_pool(name="attn_in", bufs=1)
    KT = attn_in.tile([P, HP, S], BF16)            # K^T, head pairs on partitions
    Vaug = attn_in.tile([P, KB, H, DK + 1], BF16)  # V + ones column per head
    QT = attn_in.tile([P, HP, SQ], BF16)
    # only the mask quadrants that aren't structurally trivial: with the
    # snake slot pairing, just the last MW key blocks of each slot can hold
    # a diagonal (or an all-masked block) on some core
    mS = attn_in.tile([P, SLOTS, MW, SS], BF16)
    mr = maskT.rearrange("(kb p) q -> p kb q", p=P)
    nc.vector.memset(Vaug[:, :, :, DK : DK + 1], 1.0)

    # ---- post-attention inputs (right-side stack, phase-scoped) -------
    mid_ctx = tc.alloc_tile_pool(name="mid_ctx", bufs=1, side="right")
    ctxT = mid_ctx.tile([P, DC, SQ], BF16)

    kv_in = tc.alloc_tile_pool(name="kv_in", bufs=1, side="right")
    wk_sb = kv_in.tile([P, DC, D], BF16)
    wv_sb = kv_in.tile([P, DC, D], BF16)
    xT23 = kv_in.tile([P, DC, S // 2], BF16)
    xt01p = tc.alloc_tile_pool(name="xt01p", bufs=1, side="right")
    xT01 = xt01p.tile([P, DC, S // 2], BF16)
    xTr = xT.rearrange("(c p) s -> p c s", p=P)

    def xT_at(sb):
        t = xT01 if sb < 2 else xT23
        return t, (sb % 2) * QBS

    q_in = tc.alloc_tile_pool(name="q_in", bufs=1, side="right")
    wq_sb = q_in.tile([P, DC, D], BF16)
    xqT_sb = q_in.tile([P, DC, SQ], BF16)
    # DMA order tracks first consumption (q projections, then K slab 0, ...).
    # Weight tensors go as whole contiguous transfers: slicing them per
    # column block explodes the descriptor-issue time on the queue engine.
    wkr = wk.rearrange("(c p) n -> p c n", p=P)
    wvr = wv.rearrange("(c p) n -> p c n", p=P)
    xqr = xqT.rearrange("(c p) s -> p c s", p=P)
    nc.sync.dma_start(out=wq_sb, in_=wq.rearrange("(c p) n -> p c n", p=P))
    nc.scalar.dma_start(out=xqT_sb[:, :, 0:QBS], in_=xqr[:, :, 0:QBS])
    nc.sync.dma_start(out=xT01[:, :, 0:QBS], in_=xTr[:, :, 0:QBS])
    for c in range(DC):
        nc.scalar.dma_start(out=wk_sb[:, c, :], in_=wkr[:, c, :])
    nc.sync.dma_start(out=xT01[:, :, QBS : 2 * QBS], in_=xTr[:, :, QBS : 2 * QBS])
    nc.scalar.dma_start(out=xqT_sb[:, :, QBS : 2 * QBS],
                        in_=xqr[:, :, QBS : 2 * QBS])
    for c in range(DC):
        nc.gpsimd.dma_start(out=wv_sb[:, c, :], in_=wvr[:, c, :])
    for sb in (2, 3):
        nc.gpsimd.dma_start(out=xT23[:, :, (sb % 2) * QBS : (sb % 2 + 1) * QBS],
                            in_=xTr[:, :, sb * QBS : (sb + 1) * QBS])

    # ---- LayerNorm helpers (stats on DVE, rstd batched on ACT) --------
    def ln_stats(src, mvs, j):
        # bn stats over the free dim (768) of fp32 src [128, 768]
        stats = ln_wk.tile([P, 3, 6], F32, tag="stats", bufs=3)
        for i in range(3):
            nc.vector.bn_stats(out=stats[:, i, :], in_=src[:, i * 256 : (i + 1) * 256])
        nc.vector.bn_aggr(out=mvs[:, j, :], in_=stats)

    def ln_rstd_batch(mvs, rstds, n):
        # one ACT Sqrt + one fast DVE reciprocal serve n LayerNorms
        stds = ln_wk.tile([P, 4], F32, tag="stds", bufs=2)
        nc.scalar.activation(out=stds[:, 0:n], in_=mvs[:, 0:n, 1:2],
                             func=AF.Sqrt, bias=eps_t[:, 0:1])
        nc.vector.reciprocal_approx_fast(out=rstds[:, 0:n], in_=stds[:, 0:n])

    def ln_apply(src, mvs, rstds, j, gb, bb, dst):
        nc.vector.tensor_scalar_sub(out=src, in0=src, scalar1=mvs[:, j, 0:1])
        nc.vector.scalar_tensor_tensor(out=src, in0=src, scalar=rstds[:, j : j + 1],
                                       in1=gb, op0=OP.mult, op1=OP.mult)
        nc.vector.tensor_tensor(out=dst, in0=src, in1=bb, op=OP.add)

    proj_ps = tc.alloc_tile_pool(name="proj_ps", bufs=2, space="PSUM", side="right")
    if True:
        sc_ps = tc.alloc_tile_pool(name="sc_ps", bufs=2, space="PSUM")
        cx_ps = tc.alloc_tile_pool(name="cx_ps", bufs=1, space="PSUM")
        # at_sb/nm_sb/mT0 are allocated only once the q-projection inputs are
        # released -- their SBUF footprints must not overlap
        pools = {}

        # ---------- projection work units ----------
        def q_unit(hp, sb):
            ps = proj_ps.tile([P, QBS], F32, tag="proj")
            for c in range(DC):
                nc.tensor.matmul(
                    ps, lhsT=wq_sb[:, c, hp * P : (hp + 1) * P],
                    rhs=xqT_sb[:, c, sb * QBS : (sb + 1) * QBS],
                    start=(c == 0), stop=(c == DC - 1),
                )
            nc.scalar.activation(
                out=QT[:, hp, sb * QBS : (sb + 1) * QBS], in_=ps,
                func=AF.Identity, bias=bqp[:, hp : hp + 1],
            )

        def k_unit(hp, sb, on_act=True):
            xt, off = xT_at(sb)
            ps = proj_ps.tile([P, QBS], F32, tag="proj")
            for c in range(DC):
                nc.tensor.matmul(
                    ps, lhsT=wk_sb[:, c, hp * P : (hp + 1) * P],
                    rhs=xt[:, c, off : off + QBS],
                    start=(c == 0), stop=(c == DC - 1),
                )
            if on_act:
                nc.scalar.activation(
                    out=KT[:, hp, sb * QBS : (sb + 1) * QBS], in_=ps,
                    func=AF.Identity, bias=bkp[:, hp : hp + 1],
                )
            else:
                # inside the attention interleave ACT is the bottleneck chain
                nc.vector.tensor_scalar_add(
                    out=KT[:, hp, sb * QBS : (sb + 1) * QBS], in0=ps,
                    scalar1=bkp[:, hp : hp + 1],
                )

        def v_unit(kb, nh, on_act=True):
            xt, off = xT_at(kb // (QBS // P))
            kb_off = off // P + kb % (QBS // P)
            ps = proj_ps.tile([P, QBS], F32, tag="proj")
            psv = ps[:, 0:NHW]
            for c in range(DC):
                nc.tensor.matmul(
                    psv, lhsT=xt[:, c, kb_off * P : (kb_off + 1) * P],
                    rhs=wv_sb[:, c, nh * NHW : (nh + 1) * NHW],
                    start=(c == 0), stop=(c == DC - 1),
                )
            if on_act:
                nc.scalar.activation(
                    out=Vaug[:, kb, nh * 6 : (nh + 1) * 6, 0:DK],
                    in_=psv.rearrange("p (h d) -> p h d", d=DK),
                    func=AF.Copy,
                )
            else:
                nc.vector.tensor_copy(
                    out=Vaug[:, kb, nh * 6 : (nh + 1) * 6, 0:DK],
                    in_=psv.rearrange("p (h d) -> p h d", d=DK),
                )

        def kv_slab(sb, on_act=True):
            for hp in range(HP):
                k_unit(hp, sb, on_act)
            for j in range(QBS // P):
                for nh in range(NH):
                    v_unit(sb * (QBS // P) + j, nh, on_act)

        # ---------- attention iteration ----------
        pending = []

        def make_norm(cxs_e, cxs_o, rec_p, hp, qs):
            def go():
                for i, (cxs, pb) in enumerate(((cxs_e, 0), (cxs_o, DK))):
                    den_b = pools['nm_sb'].tile([DK, 2 * SS], F32, tag="den_b",
                                                bufs=1)
                    nc.gpsimd.partition_broadcast(den_b, rec_p[0:1, i, :])
                    nc.vector.tensor_tensor(
                        out=ctxT[pb : pb + DK, hp, qs], in0=cxs,
                        in1=den_b, op=OP.mult,
                    )
            return go

        def attn_iter(hp, u, fill=None):
            # slot pair u covers slots {2u, 2u+1} = local query cols
            # [1024u/2, 1024u/2+512). Key block g streams 512-wide while both
            # slots need it, narrowing to the high slot's 256 afterwards --
            # exact block-causal coverage, one KT/Vaug weight load per block.
            nkb = EXTS[2 * u + 1]          # 8 / 16
            wide = EXTS[2 * u]             # 4 / 12
            qlo = u * 2 * SS
            qs = slice(qlo, qlo + 2 * SS)
            cx_e = cx_ps.tile([DK + 1, 2 * SS], F32, tag="cx_e")
            cx_o = cx_ps.tile([DK + 1, 2 * SS], F32, tag="cx_o")
            for gb in range(0, nkb, 2):
                # pt layout [P, hh, gi, q]: the head pair splits across PSUM
                # banks in sc, so exp reads/writes stay hh-major
                pt = pools['at_sb'].tile([P, 2, 2, 2 * SS], BF16, tag="pt")
                for gi in range(2):
                    g = gb + gi
                    ks = slice(g * P, (g + 1) * P)
                    w = 2 * SS if g < wide else SS
                    off = 0 if g < wide else SS
                    sc = sc_ps.tile([P, 2, 2 * SS], F32, tag="sc")
                    # the two heads of a pair hit disjoint PE row groups (and
                    # separate PSUM banks) and run concurrently in the array
                    nc.tensor.matmul(sc[:, 0, 0:w], lhsT=KT[0:DK, hp, ks],
                                     rhs=QT[0:DK, hp, qlo + off : qlo + 2 * SS],
                                     start=True, stop=True)
                    nc.tensor.matmul(sc[:, 1, 0:w], lhsT=KT[DK:P, hp, ks],
                                     rhs=QT[DK:P, hp, qlo + off : qlo + 2 * SS],
                                     start=True, stop=True)
                    nc.scalar.activation(out=pt[:, :, gi, off : 2 * SS],
                                         in_=sc[:, :, 0:w],
                                         func=AF.Exp, scale=1.0 / 8.0)
                # mask windows: the pair's low slot masks its last MW wide
                # blocks on the low 256 columns; the high slot masks its last
                # MW (narrow) blocks. Everything else is causally full.
                lo_s, hi_s = 2 * u, 2 * u + 1
                if wide - MW <= gb < wide:
                    mq = mS[:, lo_s, gb - (wide - MW) : gb - (wide - MW) + 2, :]
                    for hh in range(2):
                        nc.vector.tensor_tensor(
                            out=pt[:, hh, :, 0:SS], in0=pt[:, hh, :, 0:SS],
                            in1=mq, op=OP.mult,
                        )
                if gb >= nkb - MW:
                    mq = mS[:, hi_s, gb - (nkb - MW) : gb - (nkb - MW) + 2, :]
                    for hh in range(2):
                        nc.vector.tensor_tensor(
                            out=pt[:, hh, :, SS : 2 * SS],
                            in0=pt[:, hh, :, SS : 2 * SS],
                            in1=mq, op=OP.mult,
                        )
                for gi in range(2):
                    g = gb + gi
                    w = 2 * SS if g < wide else SS
                    off = 0 if g < wide else SS
                    nc.tensor.matmul(cx_e[:, off : 2 * SS],
                                     lhsT=Vaug[:, g, 2 * hp, :],
                                     rhs=pt[:, 0, gi, off : 2 * SS],
                                     start=(g == 0), stop=(g == nkb - 1),
                                     skip_group_check=True)
                    nc.tensor.matmul(cx_o[:, off : 2 * SS],
                                     lhsT=Vaug[:, g, 2 * hp + 1, :],
                                     rhs=pt[:, 1, gi, off : 2 * SS],
                                     start=(g == 0), stop=(g == nkb - 1),
                                     skip_group_check=True)
                if gb == 2 and pending:
                    # previous iteration's normalize: emitted after this
                    # iteration's first blocks so the DVE reciprocal never
                    # delays the mask multiplies
                    pending.pop()()
                # the PE is in-order: filler matmuls only absorb the exp-wait
                # bubbles if they are woven BETWEEN key-block groups
                if fill and (u == 0 or gb % 4 == 2):
                    fill.pop(0)()
            # stage ctx to SBUF immediately: frees the PSUM bank within one
            # DVE copy so the cx pool gets away with a single buffer; the
            # denominator rows pair up in one base-0 tile for one reciprocal
            cxs_e = pools['nm_sb'].tile([DK, 2 * SS], F32, tag="cxs_e")
            nc.vector.tensor_copy(out=cxs_e, in_=cx_e[0:DK, :])
            cxs_o = pools['nm_sb'].tile([DK, 2 * SS], F32, tag="cxs_o")
            nc.vector.tensor_copy(out=cxs_o, in_=cx_o[0:DK, :])
            den_p = pools['nm_sb'].tile([1, 2, 2 * SS], F32, tag="den_p")
            nc.vector.tensor_copy(out=den_p[:, 0, :], in_=cx_e[DK : DK + 1, :])
            nc.vector.tensor_copy(out=den_p[:, 1, :], in_=cx_o[DK : DK + 1, :])
            rec_p = pools['nm_sb'].tile([1, 2, 2 * SS], F32, tag="rec_p")
            nc.vector.reciprocal_approx_fast(out=rec_p, in_=den_p)
            pending.append(make_norm(cxs_e, cxs_o, rec_p, hp, qs))

        # ---------- schedule: projections + qb0 attention ----------
        for hp in range(HP):
            q_unit(hp, 0)
        kv_slab(0)
        kv_slab(1)
        for hp in range(HP):
            q_unit(hp, 1)
        q_in.release()
        xt01p.release()
        pools['at_sb'] = tc.alloc_tile_pool(name="at_sb", bufs=2)
        pools['nm_sb'] = tc.alloc_tile_pool(name="nm_sb", bufs=2)
        # mask windows load after the startup burst, in the early attention
        # window's spare DMA bandwidth (slot s is consumed ~s quarters in)
        for s in range(SLOTS):
            nc.gpsimd.dma_start(
                out=mS[:, s, :, :],
                in_=mr[:, EXTS[s] - MW : EXTS[s], s * SS : (s + 1) * SS])
        # epilogues of woven fillers split across ACT and DVE to balance the
        # two against the exp chain (ACT) and mask multiplies (DVE)
        kv_fill = [(lambda hp=hp, sb=sb: k_unit(hp, sb, on_act=False))
                   for sb in (2, 3) for hp in range(HP)] + \
                  [(lambda kb=kb, nh=nh: v_unit(kb, nh, on_act=(nh == 0)))
                   for kb in range(8, KB) for nh in range(NH)]
        for hp in range(HP):
            attn_iter(hp, 0, kv_fill)
        for fn in kv_fill:
            fn()
        kv_fill.clear()
        kv_in.release()
        proj_ps.release()

        # ---------- qb1 attention with qb0 epilogue woven in ----------
        mid_ow = tc.alloc_tile_pool(name="mid_ow", bufs=1, side="right")
        xres_sb = mid_ow.tile([P, QC, D], BF16)
        nc.gpsimd.dma_start(out=xres_sb,
                            in_=xres.rearrange("(c p) n -> p c n", p=P))
        wo_sb = mid_ow.tile([P, DC, D], BF16)
        nc.gpsimd.dma_start(out=wo_sb, in_=wo.rearrange("(c p) n -> p c n", p=P))
        op_ps = tc.alloc_tile_pool(name="op_ps", bufs=1, space="PSUM", side="right")
        tp_ps = tc.alloc_tile_pool(name="tp_ps", bufs=1, space="PSUM", side="right")

        hpre_map = {}

        def op_half(qc, nh):
            def go():
                if qc not in hpre_map:
                    hpre_map[qc] = ln_wk.tile([P, D], F32, tag="hpre",
                                              bufs=4, name=f"hpre_{qc}")
                hpre = hpre_map[qc]
                ps = op_ps.tile([P, NHW], F32, tag="op")
                for c in range(DC):
                    nc.tensor.matmul(
                        ps, lhsT=ctxT[:, c, qc * P : (qc + 1) * P],
                        rhs=wo_sb[:, c, nh * NHW : (nh + 1) * NHW],
                        start=(c == 0), stop=(c == DC - 1),
                    )
                nc.vector.scalar_tensor_tensor(
                    out=hpre[:, nh * NHW : (nh + 1) * NHW], in0=ps,
                    scalar=1.0, in1=xres_sb[:, qc, nh * NHW : (nh + 1) * NHW],
                    op0=OP.mult, op1=OP.add,
                )
            return go

        def ln1_group(qcs):
            # per-group LN state: stats land in mvs, one ACT computes all
            # rstds, applies run per qc
            mvs = ln_wk.tile([P, 4, 2], F32, tag="mvs", bufs=2)
            rstds = ln_wk.tile([P, 4], F32, tag="rstds", bufs=2)

            def st(j):
                def go():
                    ln_stats(hpre_map[qcs[j]], mvs, j)
                return go

            def rs():
                def go():
                    ln_rstd_batch(mvs, rstds, len(qcs))
                return go

            def ap(j):
                def go():
                    ln_apply(hpre_map.pop(qcs[j]), mvs, rstds, j, g1b, be1b,
                             h_sb[:, qcs[j], :])
                return go

            return [st(j) for j in range(len(qcs))], rs(), \
                   [ap(j) for j in range(len(qcs))]

        def transp_half(qc, lo):
            def go():
                tp = tp_ps.tile([P, DC // 2, P], BF16, tag="tp")
                for i, c in enumerate(range(lo, lo + DC // 2)):
                    nc.tensor.transpose(tp[:, i, :],
                                        h_sb[:, qc, c * P : (c + 1) * P], ident)
                nc.vector.tensor_copy(
                    out=hT[:, lo : lo + DC // 2, qc * P : (qc + 1) * P], in_=tp)
            return go

        st03, rs03, ap03 = ln1_group([0, 1, 2, 3])
        fillers = []
        for qc in range(4):
            fillers += [op_half(qc, 0), op_half(qc, 1), st03[qc]]
        fillers.append(rs03)
        fillers += ap03
        for qc in range(4):
            fillers += [transp_half(qc, 0), transp_half(qc, DC // 2)]
        for hp in range(HP):
            attn_iter(hp, 1, fillers)
        for fn in pending:
            fn()
        pending.clear()
        for fn in fillers:
            fn()

        # ---------- attention PSUM closes; FFN weights start streaming NOW
        # (9.2MB issued before the qc4-7 epilogue so FFN1 never stalls) ----
        cx_ps.release()
        sc_ps.release()
        pools['nm_sb'].release()
        pools['at_sb'].release()
        attn_in.release()
        w12_in = tc.alloc_tile_pool(name="w12_in", bufs=1)
        w1_sb = w12_in.tile([P, DC, DF], BF16)
        w2_sb = w12_in.tile([P, DFC, D], BF16)
        w1r = w1.rearrange("(c p) n -> p c n", p=P)
        for fq in range(4):
            nc.sync.dma_start(out=w1_sb[:, :, fq * (DF // 4) : (fq + 1) * (DF // 4)],
                              in_=w1r[:, :, fq * (DF // 4) : (fq + 1) * (DF // 4)])
        nc.scalar.dma_start(out=w2_sb, in_=w2.rearrange("(c p) n -> p c n", p=P))
        f1_ps = tc.alloc_tile_pool(name="f1_ps", bufs=3, space="PSUM")
        f1g_sb = tc.alloc_tile_pool(name="f1g_sb", bufs=1)

        def ffn1_unit(f1g, qs, f):
            ps = f1_ps.tile([P, QBS], F32, tag="f1")
            for c in range(DC):
                nc.tensor.matmul(
                    ps, lhsT=w1_sb[:, c, f * P : (f + 1) * P],
                    rhs=hT[:, c, qs], start=(c == 0), stop=(c == DC - 1),
                )
            nc.scalar.activation(out=f1g[:, f, :], in_=ps, func=AF.Gelu,
                                 bias=b1p[:, f : f + 1])

        # ---------- rest of out-projection + LN1 + transposes, interleaved
        # with FFN1 on the first query block ----------
        st47, rs47, ap47 = ln1_group([4, 5, 6, 7])
        epi = []
        for j, qc in enumerate(range(4, QC)):
            epi += [op_half(qc, 0), op_half(qc, 1), st47[j]]
        epi.append(rs47)
        epi += ap47
        for qc in range(4, QC):
            epi += [transp_half(qc, 0), transp_half(qc, DC // 2)]
        f1g0 = f1g_sb.tile([P, DFC, QBS], BF16, tag="f1g")
        for f in range(DFC):
            if epi:
                epi.pop(0)()
            ffn1_unit(f1g0, slice(0, QBS), f)
        for fn in epi:
            fn()

    tp_ps.release()
    op_ps.release()
    mid_ow.release()
    mid_ctx.release()

    # ====== FFN: f1^T = gelu(w1^T h^T + b1); out = LN2(f1g^T w2 + h) ====
    with tc.tile_pool(name="f2_ps", bufs=3, space="PSUM") as f2_ps, \
         tc.tile_pool(name="out_sb", bufs=3) as out_sb:
        def ffn2_qc(f1g, qb, sq):
            qc = qb * (QBS // P) + sq
            ot = out_sb.tile([P, D], F32, tag="ot")
            for nh in range(NH):
                ps = f2_ps.tile([P, NHW], F32, tag="f2")
                for f in range(DFC):
                    nc.tensor.matmul(
                        ps, lhsT=f1g[:, f, sq * P : (sq + 1) * P],
                        rhs=w2_sb[:, f, nh * NHW : (nh + 1) * NHW],
                        start=(f == 0), stop=(f == DFC - 1),
                    )
                nc.vector.scalar_tensor_tensor(
                    out=ot[:, nh * NHW : (nh + 1) * NHW], in0=ps, scalar=1.0,
                    in1=h_sb[:, qc, nh * NHW : (nh + 1) * NHW],
                    op0=OP.mult, op1=OP.add,
                )
            nc.vector.tensor_tensor(out=ot, in0=ot, in1=b2b, op=OP.add)
            mvs2 = ln_wk.tile([P, 4, 2], F32, tag="mvs", bufs=2)
            rstds2 = ln_wk.tile([P, 4], F32, tag="rstds", bufs=2)
            ln_stats(ot, mvs2, 0)
            ln_rstd_batch(mvs2, rstds2, 1)
            ln_apply(ot, mvs2, rstds2, 0, g2b, be2b, ot)
            nc.sync.dma_start(out=out[qc * P : (qc + 1) * P, :], in_=ot)

        for sq in range(QBS // P):
            ffn2_qc(f1g0, 0, sq)
        f1g1 = f1g_sb.tile([P, DFC, QBS], BF16, tag="f1g")
        for f in range(DFC):
            ffn1_unit(f1g1, slice(QBS, 2 * QBS), f)
        for sq in range(QBS // P):
            ffn2_qc(f1g1, 1, sq)

    f1_ps.release()
    f1g_sb.release()
    w12_in.release()
    ln_wk.release()
    ffn.release()


def build_program():
    nc = bacc.Bacc("TRN2", target_bir_lowering=False, debug=False,
                   enable_asserts=False, num_devices=N_CORES)
    io = {}

    def din(name, shape, dt):
        io[name] = nc.dram_tensor(name, list(shape), dt, kind="ExternalInput").ap()

    din("xT", (D, S), BF16)
    din("xqT", (D, SQ), BF16)
    din("xres", (SQ, D), BF16)
    din("maskT", (S, SQ), BF16)
    din("wq", (D, D), BF16)
    din("wk", (D, D), BF16)
    din("wv", (D, D), BF16)
    din("wo", (D, D), BF16)
    din("w1", (D, DF), BF16)
    din("w2", (DF, D), BF16)
    din("bqp", (P, HP), F32)
    din("bkp", (P, HP), F32)
    din("b1p", (P, DFC), F32)
    for n in ["g1r", "be1r", "g2r", "be2r", "b2r"]:
        din(n, (1, D), F32)
    io["out"] = nc.dram_tensor("out", [SQ, D], F32, kind="ExternalOutput").ap()

    with tile.TileContext(nc) as tc:
        with ExitStack() as ctx:
            emit(ctx, tc, io)
    nc.compile()
    return nc


_NC = None


def _get_program():
    global _NC
    if _NC is None:
        _NC = build_program()
    return _NC


def _qrows(half):
    # snake pairing of 256-row blocks: slot extents become [4,8,12,16] key
    # blocks on BOTH cores of a pair (exact block-causal coverage, SPMD-safe)
    blocks = [0, 3, 4, 7] if half == 0 else [1, 2, 5, 6]
    return np.concatenate([np.arange(256 * a, 256 * (a + 1)) for a in blocks])


def shard_inputs(inputs):
    x = np.asarray(inputs["x"], np.float32)
    mask = np.asarray(inputs["mask"], bool)
    w = {k: np.asarray(inputs[k], np.float32) for k in
         ["wq", "bq", "wk", "bk", "wv", "bv", "wo", "bo", "g1", "be1",
          "w1", "b1", "w2", "b2", "g2", "be2"]}

    base = dict(
        wq=np.ascontiguousarray(w["wq"].astype(BF)),
        wk=np.ascontiguousarray(w["wk"].astype(BF)),
        wv=np.ascontiguousarray(w["wv"].astype(BF)),
        wo=np.ascontiguousarray(w["wo"].astype(BF)),
        w1=np.ascontiguousarray(w["w1"].astype(BF)),
        w2=np.ascontiguousarray(w["w2"].astype(BF)),
        bqp=np.ascontiguousarray(w["bq"].reshape(HP, P).T),
        bkp=np.ascontiguousarray(w["bk"].reshape(HP, P).T),
        b1p=np.ascontiguousarray(w["b1"].reshape(DFC, P).T),
        g1r=np.ascontiguousarray(w["g1"].reshape(1, D)),
        be1r=np.ascontiguousarray(w["be1"].reshape(1, D)),
        g2r=np.ascontiguousarray(w["g2"].reshape(1, D)),
        be2r=np.ascontiguousarray(w["be2"].reshape(1, D)),
        b2r=np.ascontiguousarray(w["b2"].reshape(1, D)),
    )
    # bv and bo fold into the residual: ctx@wo + bo + x with v-bias bv adds
    # a constant row bv@wo (softmax rows sum to 1)
    res_const = (w["bo"] + w["bv"] @ w["wo"]).astype(np.float32)

    in_maps = []
    for c in range(N_CORES):
        b, half = divmod(c, 2)
        qr = _qrows(half)
        xb = x[b]
        xq = xb[qr]
        m = dict(base)
        m["xT"] = np.ascontiguousarray(xb.T.astype(BF))
        m["xqT"] = np.ascontiguousarray(xq.T.astype(BF))
        m["xres"] = np.ascontiguousarray((xq + res_const[None, :]).astype(BF))
        m["maskT"] = np.ascontiguousarray((~mask[b][qr]).T.astype(BF))
        in_maps.append(m)
    return in_maps


def gather_outputs(results):
    y = np.empty((B, S, D), np.float32)
    for c in range(N_CORES):
        b, half = divmod(c, 2)
        y[b, _qrows(half)] = results[c]["out"]
    return y


def kernel(**inputs):
    nc = _get_program()
    in_maps = shard_inputs(inputs)
    res = run_bass_kernel_spmd(nc, in_maps, list(range(N_CORES)))
    return gather_outputs(res.results)


if __name__ == "__main__":
    build_program()
    print("program built ok")
